# revision 1
# baseline (speedup 1.0000x reference)
"""Trainium2 Bass kernel: greedy bbox-matching loss (nn_BboxLoss).

Full computation: L[t,p] = pairwise bbox loss (IoU / MSE mix), then greedy
per-target argmin over still-available preds, mean of selected losses.

Strategy (8 NeuronCores, preds sharded 8 x 1024):
  device: per core, compute negated-loss tiles [128 targets x 1024 preds]
          entirely in SBUF, then top-8 per target via max/max_index.
          - PE matmul (K=6) produces -mse directly in PSUM (coords cross-term
            plus folded -sq_p/4 and -sq_t/4 rows).
          - ACT computes the min(corner) terms via warm-table Relu chains.
          - DVE does the remaining elementwise passes + top-8 extraction.
  host:   merge the 8x8=64 candidates per target, run the (inherently
          sequential, O(T*64)) greedy walk with an exactness safety check;
          rare unsafe rows fall back to a full-row recompute.

Device returns v = iou (overlap) or -mse (no overlap); true loss = 1 - v.
"""
import numpy as np
from contextlib import ExitStack

P_TOTAL = 8192
T = 2048
N_CORES = 8
NP_SHARD = 4                  # pred shards
NT_SHARD = 2                  # target shards
P_CORE = P_TOTAL // NP_SHARD  # 2048 preds per core
T_CORE = T // NT_SHARD        # 1024 targets per core
NJ = T_CORE // 128            # 8 row tiles of 128 targets
EPS = 1e-7
TOPK = 8

_CACHE = {}


def _build_nc():
    import concourse.bacc as bacc
    import concourse.mybir as mybir
    from concourse.tile import TileContext

    f32 = mybir.dt.float32
    u32 = mybir.dt.uint32
    i32 = mybir.dt.int32
    Alu = mybir.AluOpType
    Act = mybir.ActivationFunctionType

    nc = bacc.Bacc()
    ps_d = nc.dram_tensor("pshard", [1, 5 * P_CORE], f32, kind="ExternalInput")
    hp_d = nc.dram_tensor("hpredT", [6, P_CORE], f32, kind="ExternalInput")
    tsc_d = nc.dram_tensor("tscal", [128, 5 * NJ], f32, kind="ExternalInput")
    tt_d = nc.dram_tensor("tgtT", [6, T_CORE], f32, kind="ExternalInput")
    vals_d = nc.dram_tensor("cand_vals", [128, NJ * TOPK], f32, kind="ExternalOutput")
    idx_d = nc.dram_tensor("cand_idx", [128, NJ * TOPK], u32, kind="ExternalOutput")

    with TileContext(nc) as tc, ExitStack() as ctx:
        const = ctx.enter_context(tc.tile_pool(name="const", bufs=1))
        work = ctx.enter_context(tc.tile_pool(name="work", bufs=2))
        psum = ctx.enter_context(
            tc.tile_pool(name="psum", bufs=2, space="PSUM")
        )

        HP = const.tile([6, P_CORE], f32)
        TSC = const.tile([128, 5, NJ], f32)
        TT6 = const.tile([6, T_CORE], f32)
        PLANES = const.tile([128, 5, P_CORE], f32)
        VALS = const.tile([128, NJ, TOPK], f32)
        IDX = const.tile([128, NJ, TOPK], u32)

        nc.sync.dma_start(HP[:], hp_d[:])
        nc.sync.dma_start(TSC[:].rearrange("p q j -> p (q j)"), tsc_d[:])
        nc.sync.dma_start(TT6[:], tt_d[:])

        # load the five per-pred rows into partition 0 of PLANES, then
        # replicate across all 128 partitions in-place, one plane at a time,
        # ordered by first use so compute can start early
        TINY = const.tile([128, 1], f32)
        nc.vector.memset(TINY[:], 2e-38)
        PLF = PLANES[:].rearrange("p q n -> p (q n)")
        nc.sync.dma_start(PLF[0:1, :], ps_d[:])
        for q in (1, 3, 0, 2, 4):   # X1P, Y1P, X0P, Y0P, AREAP
            nc.gpsimd.partition_broadcast(
                PLANES[:, q, :], PLANES[0:1, q, :]
            )

        X0P = PLANES[:, 0, :]
        X1P = PLANES[:, 1, :]
        Y0P = PLANES[:, 2, :]
        Y1P = PLANES[:, 3, :]
        AREAP = PLANES[:, 4, :]

        for j in range(NJ):
            x0t = TSC[:, 0, j : j + 1]
            x1t = TSC[:, 1, j : j + 1]
            y0t = TSC[:, 2, j : j + 1]
            y1t = TSC[:, 3, j : j + 1]
            ate = TSC[:, 4, j : j + 1]   # area_t + EPS

            # nl1 = cross/2 - sq_t/4 - sq_p/4 = -mse, straight out of PE
            nl1 = psum.tile([128, P_CORE], f32, tag="nl1")
            lhsT = TT6[:, j * 128 : (j + 1) * 128]
            for h in range(P_CORE // 512):
                nc.tensor.matmul(
                    nl1[:, h * 512 : (h + 1) * 512],
                    lhsT,
                    HP[:, h * 512 : (h + 1) * 512],
                    start=True,
                    stop=True,
                )

            ox1 = work.tile([128, P_CORE], f32, tag="ox1")
            oy1 = work.tile([128, P_CORE], f32, tag="oy1")
            ndx = work.tile([128, P_CORE], f32, tag="ndx")
            ndy = work.tile([128, P_CORE], f32, tag="ndy")
            inter = work.tile([128, P_CORE], f32, tag="inter")
            nmr = work.tile([128, P_CORE], f32, tag="nmr")
            lnis = work.tile([128, P_CORE], f32, tag="lnis")
            lnS = work.tile([128, P_CORE], f32, tag="lnS")
            fin = work.tile([128, P_CORE], f32, tag="fin")

            # ox1 = min(x1p, x1t) = relu(x1t - relu(x1t - x1p)); warm Relu table
            nc.scalar.activation(ox1[:], X1P, Act.Relu, bias=x1t, scale=-1.0)
            nc.scalar.activation(ox1[:], ox1[:], Act.Relu, bias=x1t, scale=-1.0)
            nc.scalar.activation(oy1[:], Y1P, Act.Relu, bias=y1t, scale=-1.0)
            nc.scalar.activation(oy1[:], oy1[:], Act.Relu, bias=y1t, scale=-1.0)

            # ndx = max(x0p, x0t) - ox1  (= -dx)
            nc.vector.scalar_tensor_tensor(
                ndx[:], X0P, x0t, ox1[:], op0=Alu.max, op1=Alu.subtract
            )
            nc.vector.scalar_tensor_tensor(
                ndy[:], Y0P, y0t, oy1[:], op0=Alu.max, op1=Alu.subtract
            )
            # nmr = relu(max(ndx, ndy)): bit pattern nonzero <=> no overlap
            # inter = ndx*ndy (= dx*dy)
            nc.vector.tensor_tensor(inter[:], ndx[:], ndy[:], op=Alu.mult)
            nc.vector.scalar_tensor_tensor(
                nmr[:], ndx[:], 0.0, ndy[:], op0=Alu.max, op1=Alu.max
            )
            # log-domain rank key: ln(inter + 2e-38) - ln(area_p + area_t + EPS)
            # == monotone transform of iou (ln(iou/(1+iou))); both Ln and Relu
            # live in the natural_log act func set (one table, stays warm).
            # the -mse branch is shifted by -128 (folded into the matmul) so
            # every overlap key (>= ln(2e-38/2) ~ -88) outranks every
            # non-overlap key (<= -128); NaNs from negative inter only occur
            # at non-overlap positions, which copy_predicated overwrites.
            nc.scalar.activation(lnis[:], inter[:], Act.Ln, bias=TINY[:, 0:1])
            nc.scalar.activation(lnS[:], AREAP, Act.Ln, bias=ate)
            nc.vector.tensor_tensor(fin[:], lnis[:], lnS[:], op=Alu.subtract)
            # where no overlap, take -mse from PSUM
            nc.vector.copy_predicated(fin[:], nmr[:].bitcast(i32), nl1[:])

            nc.vector.max(out=VALS[:, j, :], in_=fin[:])
            nc.vector.max_index(IDX[:, j, :], VALS[:, j, :], fin[:])

        nc.sync.dma_start(vals_d[:], VALS[:].rearrange("p j k -> p (j k)"))
        nc.sync.dma_start(idx_d[:], IDX[:].rearrange("p j k -> p (j k)"))

    nc.compile()
    return nc


def _prep_core_inputs(pred, tgt):
    """Host-side O(P+T) derived quantities. pred [P,4], tgt [T,4] float32."""
    shared = {}
    x0t = tgt[:, 0] - tgt[:, 2] / 2
    x1t = tgt[:, 0] + tgt[:, 2] / 2
    y0t = tgt[:, 1] - tgt[:, 3] / 2
    y1t = tgt[:, 1] + tgt[:, 3] / 2
    ate = tgt[:, 2] * tgt[:, 3] + np.float32(EPS)
    qt4 = np.sum(tgt * tgt, axis=-1) / 4
    ones_t = np.ones_like(qt4)
    tscal = np.stack([x0t, x1t, y0t, y1t, ate]).astype(np.float32)  # [5, T]
    shared["tscal"] = tscal
    shared["tgtT"] = np.ascontiguousarray(
        np.concatenate(
            [tgt.T, ones_t[None, :], -(qt4 + np.float32(128.0))[None, :]]
        ).astype(np.float32)
    )

    in_maps = []
    for c in range(N_CORES):
        px = c % NP_SHARD
        sh = pred[px * P_CORE : (px + 1) * P_CORE]
        x0p = np.maximum(sh[:, 0] - sh[:, 2] / 2, np.float32(0.0))
        x1p = np.minimum(sh[:, 0] + sh[:, 2] / 2, np.float32(1.0))
        y0p = np.maximum(sh[:, 1] - sh[:, 3] / 2, np.float32(0.0))
        y1p = np.minimum(sh[:, 1] + sh[:, 3] / 2, np.float32(1.0))
        areap = sh[:, 2] * sh[:, 3]
        qp = np.sum(sh * sh, axis=-1) / 4
        ones_p = np.ones_like(qp)
        ty = c // NP_SHARD
        tsl = slice(ty * T_CORE, (ty + 1) * T_CORE)
        in_maps.append(
            {
                "pshard": np.ascontiguousarray(
                    np.stack([x0p, x1p, y0p, y1p, areap]).astype(np.float32)
                ).reshape(1, 5 * P_CORE),
                "hpredT": np.ascontiguousarray(
                    np.concatenate(
                        [0.5 * sh.T, -qp[None, :], ones_p[None, :]]
                    ).astype(np.float32)
                ),
                "tscal": np.ascontiguousarray(
                    shared["tscal"][:, tsl].reshape(5, -1, 128).transpose(2, 0, 1)
                    .reshape(128, -1)
                ),
                "tgtT": np.ascontiguousarray(shared["tgtT"][:, tsl]),
            }
        )
    return in_maps


def _row_loss(pred, trow):
    """Exact device-form loss of one target row vs all preds (numpy f32)."""
    x0p = np.maximum(pred[:, 0] - pred[:, 2] / 2, np.float32(0.0))
    x1p = np.minimum(pred[:, 0] + pred[:, 2] / 2, np.float32(1.0))
    y0p = np.maximum(pred[:, 1] - pred[:, 3] / 2, np.float32(0.0))
    y1p = np.minimum(pred[:, 1] + pred[:, 3] / 2, np.float32(1.0))
    areap = pred[:, 2] * pred[:, 3]
    x0t = trow[0] - trow[2] / 2
    x1t = trow[0] + trow[2] / 2
    y0t = trow[1] - trow[3] / 2
    y1t = trow[1] + trow[3] / 2
    ndx = np.maximum(x0p, x0t) - np.minimum(x1p, x1t)
    ndy = np.maximum(y0p, y0t) - np.minimum(y1p, y1t)
    inter = ndx * ndy
    nov = np.maximum(ndx, ndy) > 0
    dneg = (inter - (trow[2] * trow[3] + np.float32(EPS))) - areap
    with np.errstate(divide="ignore", invalid="ignore"):
        iou = (-inter) * np.reciprocal(dneg)
    cross = pred @ (0.5 * trow).astype(np.float32)
    nmse = (cross - np.sum(trow * trow) / 4) - np.sum(pred * pred, axis=-1) / 4
    v = np.where(nov, nmse, iou)  # device value; loss = 1 - v
    return (np.float32(1.0) - v).astype(np.float32)


def _pair_losses(p, t):
    """Reference-form loss for matched pairs p[i] <-> t[i] (numpy f32->f64)."""
    p = p.astype(np.float32); t = t.astype(np.float32)
    x0p = np.maximum(p[:, 0] - p[:, 2] / 2, np.float32(0.0))
    x1p = np.minimum(p[:, 0] + p[:, 2] / 2, np.float32(1.0))
    y0p = np.maximum(p[:, 1] - p[:, 3] / 2, np.float32(0.0))
    y1p = np.minimum(p[:, 1] + p[:, 3] / 2, np.float32(1.0))
    x0t = t[:, 0] - t[:, 2] / 2
    x1t = t[:, 0] + t[:, 2] / 2
    y0t = t[:, 1] - t[:, 3] / 2
    y1t = t[:, 1] + t[:, 3] / 2
    ox0 = np.maximum(x0t, x0p); ox1 = np.minimum(x1t, x1p)
    oy0 = np.maximum(y0t, y0p); oy1 = np.minimum(y1t, y1p)
    nov = (ox1 < ox0) | (oy1 < oy0)
    inter = (ox1 - ox0) * (oy1 - oy0)
    denom = p[:, 2] * p[:, 3] + t[:, 2] * t[:, 3] - inter + np.float32(EPS)
    iou = inter / denom
    mse = np.sum((p - t) * (p - t), axis=-1) / np.float32(4.0)
    return np.where(nov, np.float32(1.0) + mse,
                    np.float32(1.0) - iou).astype(np.float64)


def _host_greedy(vals, idxs, pred, tgt):
    """vals/idxs [T, NP_SHARD, TOPK]: per-target candidates from each pred shard."""
    NSH = NP_SHARD
    loss = (1.0 - vals.reshape(T, NSH * TOPK).astype(np.float64))
    gidx = (
        idxs.astype(np.int64)
        + (np.arange(NSH)[None, :, None] * P_CORE)
    ).reshape(T, NSH * TOPK)

    taken = np.zeros(P_TOTAL, dtype=bool)
    sel = np.zeros(T, dtype=np.int64)
    for t in range(T):
        lt, gt = loss[t], gidx[t]
        order = np.lexsort((gt, lt))
        chosen = -1
        depth = 0
        for d in order:
            if not taken[gt[d]]:
                chosen = d
                break
            depth += 1
        safe = chosen >= 0
        if safe and depth >= TOPK:
            # a fully-taken shard whose worst listed candidate is better than
            # our choice could hide the true argmin
            closs = lt[chosen]
            for s in range(NSH):
                blk = slice(s * TOPK, (s + 1) * TOPK)
                if lt[s * TOPK + TOPK - 1] < closs and taken[gt[blk]].all():
                    safe = False
                    break
        if safe:
            k = gt[chosen]
        else:
            row = _row_loss(pred, tgt[t]).astype(np.float64)
            row[taken] = np.inf
            k = int(np.argmin(row))
        taken[k] = True
        sel[t] = k
    # exact reference-form loss of the selected pairs
    return np.float32(_pair_losses(pred[sel], tgt).mean())


def kernel(pred_bboxes, target_bboxes):
    from concourse.bass_utils import run_bass_kernel_spmd

    pred = np.asarray(pred_bboxes, dtype=np.float32)[0]
    tgt = np.asarray(target_bboxes, dtype=np.float32)[0]

    if "nc" not in _CACHE:
        _CACHE["nc"] = _build_nc()
    nc = _CACHE["nc"]

    in_maps = _prep_core_inputs(pred, tgt)
    res = run_bass_kernel_spmd(nc, in_maps, list(range(N_CORES)))
    results = res.results
    # core c covers targets [ (c//NP) * T_CORE : ... ], pred shard c % NP
    vals = np.empty((T, NP_SHARD, TOPK), np.float32)
    idxs = np.empty((T, NP_SHARD, TOPK), np.uint32)
    def _deint(a):
        # [128, NJ*TOPK] -> [T_CORE, TOPK]; target t = j*128 + p
        return (
            a.reshape(128, -1, TOPK).transpose(1, 0, 2).reshape(T_CORE, TOPK)
        )
    for c in range(N_CORES):
        px, ty = c % NP_SHARD, c // NP_SHARD
        vals[ty * T_CORE : (ty + 1) * T_CORE, px] = _deint(results[c]["cand_vals"])
        idxs[ty * T_CORE : (ty + 1) * T_CORE, px] = _deint(results[c]["cand_idx"])
    return _host_greedy(vals, idxs, pred, tgt)



# revision 7
# speedup vs baseline: 1.9639x; 1.9639x over previous
"""Trainium2 Bass kernel: greedy bbox-matching loss (nn_BboxLoss).

Full computation: L[t,p] = pairwise bbox loss (IoU / MSE mix), then greedy
per-target argmin over still-available preds, mean of selected losses.

Strategy (8 NeuronCores, preds sharded 8 x 1024, targets replicated):
  device: per core, for each 128-target row tile compute a rank key
          monotone in IoU for every (target, pred) pair and return the
          top-8 preds per target with the pred index PACKED into the low
          mantissa bits (one InstMax, no max_index):
          - corner overlap widths via tensor_scalar min/max (4x DVE mode)
            and PE identity-matmul accumulation into PSUM,
          - per-axis Ln on ACT: key = ln(m1)+ln(m2)-ln(area_p+area_t+eps)
            = ln(inter/S), monotone in IoU; non-overlap (m <= 0) becomes
            NaN which is clamped to -3e38 (junk) on Pool,
          - key assembled by f32r identity matmuls into PSUM, bitwise
            packed with a column iota on DVE, top-8 via one DVE max.
  host:   decode candidates, recompute exact reference-form losses for
          the 64 candidates per target, run the (inherently sequential)
          greedy walk with conservative full-row fallback when the
          candidate lists cannot prove the argmin.
"""
import numpy as np
from contextlib import ExitStack

P_TOTAL = 8192
T = 2048
N_CORES = 8
NP_SHARD = 8                  # pred shards (one per core)
P_CORE = P_TOTAL // NP_SHARD  # 1024 preds per core
NJ = T // 128                 # 16 row tiles of 128 targets
EPS = 1e-7
TOPK = 8
IDX_BITS = 10                 # P_CORE = 1024
IDX_MASK = (1 << IDX_BITS) - 1
KEY_MASK = 0xFFFFFFFF ^ IDX_MASK
CLAMP_VAL = -1.0e30           # junk key for non-overlap (finite: sums stay finite)
INVALID_THR = -1.0e28         # host-side validity threshold on decoded keys
MARGIN = 0.03                 # device-key approximation safety margin

_CACHE = {}


def _build_nc():
    import concourse.bacc as bacc
    import concourse.mybir as mybir
    from concourse.tile import TileContext

    f32 = mybir.dt.float32
    f32r = mybir.dt.float32r
    bf16 = mybir.dt.bfloat16
    u32 = mybir.dt.uint32
    Alu = mybir.AluOpType
    Act = mybir.ActivationFunctionType

    nc = bacc.Bacc()
    # x0p|x1p|y0p|y1p clipped corners then areap, all bf16
    pb_d = nc.dram_tensor("pshard", [1, 5 * P_CORE], bf16, kind="ExternalInput")
    tsc_d = nc.dram_tensor("tscal", [128, 5 * NJ], f32, kind="ExternalInput")
    idb_d = nc.dram_tensor("identb", [128, 256], bf16, kind="ExternalInput")
    idr_d = nc.dram_tensor("identr", [128, 256], f32r, kind="ExternalInput")
    cand_d = nc.dram_tensor("cand", [128, NJ * TOPK], u32, kind="ExternalOutput")

    with TileContext(nc) as tc, ExitStack() as ctx:
        const = ctx.enter_context(tc.tile_pool(name="const", bufs=1))
        work = ctx.enter_context(tc.tile_pool(name="work", bufs=2))
        psA = ctx.enter_context(tc.tile_pool(name="psA", bufs=1, space="PSUM"))
        psB = ctx.enter_context(tc.tile_pool(name="psB", bufs=1, space="PSUM"))
        psF = ctx.enter_context(tc.tile_pool(name="psF", bufs=2, space="PSUM"))

        PL = const.tile([128, 5, P_CORE], bf16)   # corner planes + areap
        TSC = const.tile([128, 5, NJ], f32)
        IDB = const.tile([128, 256], bf16)        # [I | -I] bf16
        IDR = const.tile([128, 256], f32r)        # [I | -I] f32r
        IOTA = const.tile([128, P_CORE], u32)
        MSKC = const.tile([128, 1], u32)
        CAND = const.tile([128, NJ, TOPK], u32)

        nc.sync.dma_start(TSC[:].rearrange("p q j -> p (q j)"), tsc_d[:])
        nc.sync.dma_start(IDB[:], idb_d[:])
        nc.sync.dma_start(IDR[:], idr_d[:])
        # replicate the five per-pred rows across partitions, ordered by use
        PLF = PL[:].rearrange("p q n -> p (q n)")
        for q in range(5):
            nc.sync.dma_start(
                PL[:, q, :],
                pb_d[:, q * P_CORE : (q + 1) * P_CORE].partition_broadcast(128),
            )
        nc.gpsimd.iota(IOTA[:], pattern=[[1, P_CORE]], base=0, channel_multiplier=0)
        nc.vector.memset(MSKC[:], KEY_MASK)

        X0P = PL[:, 0, :]
        X1P = PL[:, 1, :]
        Y0P = PL[:, 2, :]
        Y1P = PL[:, 3, :]
        AREAP = PL[:, 4, :]
        ID_P = IDB[:, 0:128]
        ID_N = IDB[:, 128:256]
        IR_P = IDR[:, 0:128]
        IR_N = IDR[:, 128:256]
        NCH = P_CORE // 512

        for j in range(NJ):
            x0t = TSC[:, 0, j : j + 1]
            x1t = TSC[:, 1, j : j + 1]
            y0t = TSC[:, 2, j : j + 1]
            y1t = TSC[:, 3, j : j + 1]
            ate = TSC[:, 4, j : j + 1]   # area_t + EPS

            CX = work.tile([128, P_CORE], bf16, tag="cx")
            MX = work.tile([128, P_CORE], bf16, tag="mx")
            CY = work.tile([128, P_CORE], bf16, tag="cy")
            MY = work.tile([128, P_CORE], bf16, tag="my")
            LNU = work.tile([128, P_CORE], f32, tag="lnu")
            LNV = work.tile([128, P_CORE], f32, tag="lnv")
            LUC = work.tile([128, P_CORE], f32r, tag="luc")
            LVC = work.tile([128, P_CORE], f32r, tag="lvc")
            LNS = work.tile([128, P_CORE], f32r, tag="lns")
            PK = work.tile([128, P_CORE], u32, tag="pk")
            M1 = psA.tile([128, P_CORE], f32, tag="m1")
            M2 = psB.tile([128, P_CORE], f32, tag="m2")
            FIN = psF.tile([128, P_CORE], f32, tag="fin")

            # corners: cx = max(x0p, x0t) etc.
            nc.vector.tensor_scalar(CX[:], X0P, x0t, None, op0=Alu.max)
            nc.vector.tensor_scalar(MX[:], X1P, x1t, None, op0=Alu.min)
            nc.vector.tensor_scalar(CY[:], Y0P, y0t, None, op0=Alu.max)
            nc.vector.tensor_scalar(MY[:], Y1P, y1t, None, op0=Alu.min)

            # overlap widths in PSUM: m1 = mx - cx, m2 = my - cy
            for h in range(NCH):
                sl = slice(h * 512, (h + 1) * 512)
                nc.tensor.matmul(M1[:, sl], ID_P, MX[:, sl], start=True, stop=False)
                nc.tensor.matmul(M1[:, sl], ID_N, CX[:, sl], start=False, stop=True)
            for h in range(NCH):
                sl = slice(h * 512, (h + 1) * 512)
                nc.tensor.matmul(M2[:, sl], ID_P, MY[:, sl], start=True, stop=False)
                nc.tensor.matmul(M2[:, sl], ID_N, CY[:, sl], start=False, stop=True)

            # ln of widths; m <= 0 -> NaN, clamped to a finite junk value on
            # Pool BEFORE the fin matmuls (0*NaN = NaN would poison columns)
            nc.scalar.activation(LNU[:], M1[:], Act.Ln)
            nc.scalar.activation(LNV[:], M2[:], Act.Ln)
            nc.gpsimd.tensor_scalar(LUC[:], LNU[:], CLAMP_VAL, None, op0=Alu.max)
            nc.gpsimd.tensor_scalar(LVC[:], LNV[:], CLAMP_VAL, None, op0=Alu.max)
            nc.scalar.activation(LNS[:], AREAP, Act.Ln, bias=ate)

            # fin = lnu + lnv - lnS  (f32r identity matmuls, PSUM accumulate)
            for h in range(NCH):
                sl = slice(h * 512, (h + 1) * 512)
                nc.tensor.matmul(FIN[:, sl], IR_P, LUC[:, sl], start=True, stop=False)
                nc.tensor.matmul(FIN[:, sl], IR_P, LVC[:, sl], start=False, stop=False)
                nc.tensor.matmul(FIN[:, sl], IR_N, LNS[:, sl], start=False, stop=True)

            # pack index into low mantissa bits; fin is always finite, so the
            # top-8 max runs directly on the packed values
            nc.vector.scalar_tensor_tensor(
                PK[:], FIN[:].bitcast(u32), MSKC[:, 0:1], IOTA[:],
                op0=Alu.bitwise_and, op1=Alu.bitwise_or,
            )
            nc.vector.max(CAND[:, j, :].bitcast(f32), PK[:].bitcast(f32))

        nc.sync.dma_start(cand_d[:], CAND[:].rearrange("p j k -> p (j k)"))

    nc.compile()
    return nc


def _prep_core_inputs(pred, tgt):
    """Host-side O(P+T) derived quantities. pred [P,4], tgt [T,4] float32."""
    try:
        import ml_dtypes
        bf = ml_dtypes.bfloat16
    except Exception:
        import jax.numpy as jnp
        bf = jnp.bfloat16

    x0t = tgt[:, 0] - tgt[:, 2] / 2
    x1t = tgt[:, 0] + tgt[:, 2] / 2
    y0t = tgt[:, 1] - tgt[:, 3] / 2
    y1t = tgt[:, 1] + tgt[:, 3] / 2
    ate = tgt[:, 2] * tgt[:, 3] + np.float32(EPS)
    tscal = np.stack([x0t, x1t, y0t, y1t, ate]).astype(np.float32)  # [5, T]
    tsc = np.ascontiguousarray(
        tscal.reshape(5, NJ, 128).transpose(2, 0, 1).reshape(128, 5 * NJ)
    )

    ident = np.eye(128, dtype=np.float32)
    idb = np.ascontiguousarray(
        np.concatenate([ident, -ident], axis=1)
    )
    idb_bf = idb.astype(bf)
    idr = idb.astype(np.float32)

    in_maps = []
    for c in range(N_CORES):
        sh = pred[c * P_CORE : (c + 1) * P_CORE]
        x0p = np.maximum(sh[:, 0] - sh[:, 2] / 2, np.float32(0.0))
        x1p = np.minimum(sh[:, 0] + sh[:, 2] / 2, np.float32(1.0))
        y0p = np.maximum(sh[:, 1] - sh[:, 3] / 2, np.float32(0.0))
        y1p = np.minimum(sh[:, 1] + sh[:, 3] / 2, np.float32(1.0))
        areap = sh[:, 2] * sh[:, 3]
        pshard = np.ascontiguousarray(
            np.stack([x0p, x1p, y0p, y1p, areap]).astype(bf).reshape(1, 5 * P_CORE)
        )
        in_maps.append(
            {
                "pshard": pshard,
                "tscal": tsc,
                "identb": idb_bf,
                "identr": idr,
            }
        )
    return in_maps


def _pair_losses(p, t):
    """Reference-form loss for matched pairs p[i] <-> t[i] (numpy f32->f64)."""
    p = p.astype(np.float32); t = t.astype(np.float32)
    x0p = np.maximum(p[:, 0] - p[:, 2] / 2, np.float32(0.0))
    x1p = np.minimum(p[:, 0] + p[:, 2] / 2, np.float32(1.0))
    y0p = np.maximum(p[:, 1] - p[:, 3] / 2, np.float32(0.0))
    y1p = np.minimum(p[:, 1] + p[:, 3] / 2, np.float32(1.0))
    x0t = t[:, 0] - t[:, 2] / 2
    x1t = t[:, 0] + t[:, 2] / 2
    y0t = t[:, 1] - t[:, 3] / 2
    y1t = t[:, 1] + t[:, 3] / 2
    ox0 = np.maximum(x0t, x0p); ox1 = np.minimum(x1t, x1p)
    oy0 = np.maximum(y0t, y0p); oy1 = np.minimum(y1t, y1p)
    nov = (ox1 < ox0) | (oy1 < oy0)
    inter = (ox1 - ox0) * (oy1 - oy0)
    denom = p[:, 2] * p[:, 3] + t[:, 2] * t[:, 3] - inter + np.float32(EPS)
    iou = inter / denom
    mse = np.sum((p - t) * (p - t), axis=-1) / np.float32(4.0)
    return np.where(nov, np.float32(1.0) + mse,
                    np.float32(1.0) - iou).astype(np.float64)


def _row_loss_ref(pred, trow):
    """Reference-form loss of one target row vs all preds (numpy f32)."""
    x0p = np.maximum(pred[:, 0] - pred[:, 2] / 2, np.float32(0.0))
    x1p = np.minimum(pred[:, 0] + pred[:, 2] / 2, np.float32(1.0))
    y0p = np.maximum(pred[:, 1] - pred[:, 3] / 2, np.float32(0.0))
    y1p = np.minimum(pred[:, 1] + pred[:, 3] / 2, np.float32(1.0))
    x0t = trow[0] - trow[2] / 2
    x1t = trow[0] + trow[2] / 2
    y0t = trow[1] - trow[3] / 2
    y1t = trow[1] + trow[3] / 2
    ox0 = np.maximum(x0t, x0p); ox1 = np.minimum(x1t, x1p)
    oy0 = np.maximum(y0t, y0p); oy1 = np.minimum(y1t, y1p)
    nov = (ox1 < ox0) | (oy1 < oy0)
    inter = (ox1 - ox0) * (oy1 - oy0)
    denom = pred[:, 2] * pred[:, 3] + trow[2] * trow[3] - inter + np.float32(EPS)
    iou = inter / denom
    d = pred - trow[None, :]
    mse = np.sum(d * d, axis=-1) / np.float32(4.0)
    return np.where(nov, np.float32(1.0) + mse, np.float32(1.0) - iou)


def _host_greedy(cand_u32, pred, tgt):
    """cand_u32 [N_CORES, T, TOPK]: packed top-8 per (target, pred shard)."""
    NSH = N_CORES
    u = cand_u32.transpose(1, 0, 2).reshape(T, NSH * TOPK)   # [T, 64]
    idx_l = (u & np.uint32(IDX_MASK)).astype(np.int64)
    shard_of = np.broadcast_to(
        np.arange(NSH, dtype=np.int64)[None, :, None], (T, NSH, TOPK)
    ).reshape(T, NSH * TOPK)
    gidx = shard_of * P_CORE + idx_l
    keyf = (u & np.uint32(KEY_MASK)).view(np.float32)
    valid = np.isfinite(keyf) & (keyf > INVALID_THR)

    # exact reference-form loss for every candidate
    tgt_rep = np.repeat(tgt, NSH * TOPK, axis=0)
    loss = _pair_losses(pred[gidx.reshape(-1)], tgt_rep).reshape(T, NSH * TOPK)
    loss[~valid] = np.inf

    nvalid = valid.reshape(T, NSH, TOPK).sum(axis=2)         # [T, NSH]
    order = np.lexsort((gidx, loss), axis=1)                 # per-row

    taken = np.zeros(P_TOTAL, dtype=bool)
    sel = np.empty(T, dtype=np.int64)
    n_fallback = 0
    for t in range(T):
        lt = loss[t]; gt = gidx[t]; ot = order[t]
        chosen = -1
        for d in ot:
            if lt[d] == np.inf:
                break
            if not taken[gt[d]]:
                chosen = d
                break
        safe = chosen >= 0
        if safe:
            closs = lt[chosen]
            # a full shard (8 listed) may hide better preds below its 8th
            # listed key; a partial shard lists ALL its overlap pairs, so it
            # only hides mse-branch pairs (loss >= 1).
            if closs >= np.float32(1.0) - MARGIN:
                safe = False
            else:
                vt = valid[t].reshape(NSH, TOPK)
                ls = lt.reshape(NSH, TOPK)
                gs = gt.reshape(NSH, TOPK)
                for s in range(NSH):
                    if nvalid[t, s] == TOPK:
                        # worst listed candidate of the full shard
                        wl = ls[s, TOPK - 1]
                        if wl < closs + MARGIN and taken[gs[s][vt[s]]].all():
                            safe = False
                            break
        if safe:
            k = gt[chosen]
        else:
            n_fallback += 1
            row = _row_loss_ref(pred, tgt[t]).astype(np.float64)
            row[taken] = np.inf
            k = int(np.argmin(row))
        taken[k] = True
        sel[t] = k
    _host_greedy.n_fallback = n_fallback
    return np.float32(_pair_losses(pred[sel], tgt).mean())


def kernel(pred_bboxes, target_bboxes):
    from concourse.bass_utils import run_bass_kernel_spmd

    pred = np.asarray(pred_bboxes, dtype=np.float32)[0]
    tgt = np.asarray(target_bboxes, dtype=np.float32)[0]

    if "nc" not in _CACHE:
        _CACHE["nc"] = _build_nc()
    nc = _CACHE["nc"]

    in_maps = _prep_core_inputs(pred, tgt)
    res = run_bass_kernel_spmd(nc, in_maps, list(range(N_CORES)))
    cand = _collect(res.results)
    return _host_greedy(cand, pred, tgt)


def _collect(results):
    """results[c]['cand'] [128, NJ*TOPK] u32 -> [N_CORES, T, TOPK]."""
    cand = np.empty((N_CORES, T, TOPK), np.uint32)
    for c in range(N_CORES):
        a = results[c]["cand"].reshape(128, NJ, TOPK)
        # target t = j*128 + p
        cand[c] = a.transpose(1, 0, 2).reshape(T, TOPK)
    return cand
